# revision 23
# baseline (speedup 1.0000x reference)
"""BiLSTM Trainium2 kernel — sequence-chunk parallel, weight-stationary,
dual-stream recurrence.

Each direction's T=2048 steps split into 8 chunks of 256, each preceded by a
W=128 step warm-up that re-converges the LSTM state (state forgets initial
conditions exponentially; validated ~8e-7 rel err).  Cores 0-3 run forward
chunks (2j, 2j+1) as two concurrent streams, cores 4-7 the backward chunks.
No cross-core communication; the host sums the two fc half-products.

The recurrence is weight-load bound: every step must stream all of W_hh into
the PE.  Two independent streams of the SAME direction share each 128x128
W_hhT stationary load — one matmul with a [128,2] moving operand (h columns
of stream A and B) computes both streams' gate chunks, halving weight traffic
per time step vs one stream.

Gates/h/c layouts are column-interleaved [k,s] (col = 2k+s, s = stream), so
every cell op is elementwise on [128,16] tiles and the h plane feeds the next
double-step's moving operands directly (rhs = h[:, 2k:2k+2]) — no transposes.
xp for both streams is seeded into PSUM via a K=2 matmul (lhsT = stacked xp
row slices, rhs = 2x2 identity).

Per-core device program (t2 = 256 + 128 = 384 double-steps):
  phase 1: xp = x @ W_ih.T + bias  (both streams)  -> DRAM [t2, 2, 4H] bf16
  phase 2: dual LSTM scan, weight-stationary       -> DRAM hs [t2, 128, 16]
  phase 3: out = hs[W:] @ fcWT (half of fc)        -> out [512, C] f32
Chunk 0 needs exact h=c=0 at its first real step: its warm-up tile gets x=0
and a zeroed first-tile bias (bias0A input).
"""

import numpy as np

T, I, H, C = 2048, 1024, 1024, 1000
FH = 4 * H
KB = H // 128    # 8 contraction chunks
GB = FH // 128   # 32 gate chunks
NCH = 32         # chunks per direction
CH = T // NCH    # 64 chunk steps
W = 16           # warm-up steps (validated: restart err 3.2e-4 at 16 steps)
T2 = CH + W      # 80 octo-steps per core
S = 8            # streams per core
U = 2            # double-steps unrolled per For_i iteration

_CACHE = {}


def _split_waits(nc):
    """walrus rejects instructions carrying more sem waits than their ISA
    encoding has slots for.  Hoist excess waits onto injected same-engine
    NOPs placed just before the instruction."""
    import concourse.mybir as mybir

    ctr = 0
    for fn in nc.m.functions:
        for bb in fn.blocks:
            insts = bb.instructions
            if not any(
                inst.sync_info is not None
                and inst.sync_info.on_wait
                and len(inst.sync_info.on_wait) > 1
                for inst in insts
            ):
                continue
            out = []
            for inst in insts:
                si = inst.sync_info
                limit = 1
                if si is not None and si.on_wait and len(si.on_wait) > limit:
                    waits = list(si.on_wait)
                    si.on_wait = waits[len(waits) - limit:]
                    for w in waits[: len(waits) - limit]:
                        nop = mybir.InstNoOp(
                            name=f"bass-waitsplit-{ctr}",
                            engine=inst.engine,
                            ins=[],
                            outs=[],
                            sync_info=mybir.SyncInfo(on_wait=[w], on_update=[]),
                        )
                        ctr += 1
                        out.append(nop)
                out.append(inst)
            insts[:] = out


def _build(t2, rep=1):
    import contextlib

    import concourse.bass as bass
    import concourse.mybir as mybir
    import concourse.tile as tile
    from concourse.bass import ds
    from concourse.masks import make_identity

    f32 = mybir.dt.float32
    bf16 = mybir.dt.bfloat16
    AF = mybir.ActivationFunctionType

    nc = bass.Bass()
    xT_d = nc.dram_tensor("xT", [I, S * t2], bf16, kind="ExternalInput")
    wihT_d = nc.dram_tensor("wihT", [I, FH], bf16, kind="ExternalInput")
    bias0A_d = nc.dram_tensor("bias0A", [1, FH], bf16, kind="ExternalInput")
    bias_d = nc.dram_tensor("bias", [1, FH], bf16, kind="ExternalInput")
    whhT_d = nc.dram_tensor("whhT", [H, FH], bf16, kind="ExternalInput")
    fcWT_d = nc.dram_tensor("fcWT", [H, C], bf16, kind="ExternalInput")
    ones_d = nc.dram_tensor("ones1", [1, 128], bf16, kind="ExternalInput")
    eyeS_d = nc.dram_tensor("eyeS", [S, S], bf16, kind="ExternalInput")
    out_d = nc.dram_tensor("out", [S * CH, C], f32, kind="ExternalOutput")

    TM = t2 // 128  # 128-row time tiles per stream

    with tile.TileContext(nc) as tc:
        ctx = contextlib.ExitStack()
        with ctx:
            xp_d = nc.dram_tensor("xp_scratch", [t2, S, FH], bf16,
                                  kind="Internal")
            hs_d = nc.dram_tensor("hs_scratch", [t2, 128, KB * S], bf16,
                                  kind="Internal")

            const = ctx.enter_context(tc.tile_pool(name="const", bufs=1))
            ident = const.tile([128, 128], bf16, tag="ident")
            make_identity(nc, ident[:])
            ones1 = const.tile([1, 128], bf16, tag="ones1")
            nc.sync.dma_start(ones1[:], ones_d[:, :])
            eyeS = const.tile([S, S], bf16, tag="eyeS")
            nc.sync.dma_start(eyeS[:], eyeS_d[:, :])

            # ------------- phase 1: xp = x @ W_ih.T + bias (both streams) ---
            with tc.tile_pool(name="p1w", bufs=1) as p1w, \
                 tc.tile_pool(name="p1", bufs=3) as p1, \
                 tc.tile_pool(name="p1ps", bufs=4, space="PSUM") as p1ps:
                wih = []
                for k in range(KB):
                    w = p1w.tile([128, FH], bf16, tag=f"wih{k}")
                    nc.sync.dma_start(w[:], wihT_d[k * 128:(k + 1) * 128, :])
                    wih.append(w)
                bias0A_sb = p1w.tile([1, FH], bf16, tag="bias0A")
                nc.sync.dma_start(bias0A_sb[:], bias0A_d[:, :])
                bias_sb = p1w.tile([1, FH], bf16, tag="bias")
                nc.sync.dma_start(bias_sb[:], bias_d[:, :])

                subs = [(0, W), (W, CH)]  # (row offset, height) per stream
                for s in range(S):
                    for off, ht in subs:
                        bsel = bias0A_sb if (off == 0 and s == 0) else (
                            bias_sb if off else bias0A_sb)
                        # off==0 tile is the warm-up tile: bias0A for stream 0
                        # (exact-zero warm-up on chunk-0 cores), bias for the
                        # rest -- host supplies bias0A=bias on non-chunk-0
                        # cores, so streams 1..3 use plain bias here.
                        if off == 0 and s > 0:
                            bsel = bias_sb
                        col = s * t2 + off
                        xt = []
                        for k in range(KB):
                            xk = p1.tile([128, 128], bf16, tag=f"xt{k}")
                            nc.sync.dma_start(
                                xk[:, 0:ht], xT_d[k * 128:(k + 1) * 128,
                                                  col:col + ht]
                            )
                            xt.append(xk)
                        for n in range(FH // 512):
                            ns = slice(n * 512, (n + 1) * 512)
                            ps = p1ps.tile([128, 512], f32, tag="ps")
                            nc.tensor.matmul(
                                ps[0:ht, :], ones1[0:1, 0:ht], bsel[0:1, ns],
                                start=True, stop=False,
                            )
                            for k in range(KB):
                                nc.tensor.matmul(
                                    ps[0:ht, :], xt[k][:, 0:ht], wih[k][:, ns],
                                    start=False, stop=(k == KB - 1),
                                )
                            xo = p1.tile([128, 512], bf16, tag="xo")
                            nc.scalar.copy(xo[0:ht, :], ps[0:ht, :])
                            nc.sync.dma_start(
                                xp_d[off:off + ht, s, ns], xo[0:ht, :]
                            )

            tc.strict_bb_all_engine_barrier()

            # ------------- phase 2: dual-stream LSTM scan -------------------
            with tc.tile_pool(name="whh", bufs=1) as whhp, \
                 tc.tile_pool(name="state", bufs=1) as state, \
                 tc.tile_pool(name="cell", bufs=2) as cell, \
                 tc.tile_pool(name="xprow", bufs=3) as xprow_pool, \
                 tc.tile_pool(name="psif", bufs=2, space="PSUM") as psif, \
                 tc.tile_pool(name="psg", bufs=2, space="PSUM") as psg, \
                 tc.tile_pool(name="pso", bufs=2, space="PSUM") as pso:
                whh = []
                for k in range(KB):
                    w = whhp.tile([128, FH], bf16, tag=f"whh{k}", name=f"whh{k}")
                    nc.sync.dma_start(w[:], whhT_d[k * 128:(k + 1) * 128, :])
                    whh.append(w)

                # h ring: plane u holds both streams' h, col = 2k + s.
                hsb = state.tile([128, U, KB * S], bf16, tag="hsb")
                nc.vector.memset(hsb[:], 0.0)
                cst = [state.tile([128, KB * S], f32, tag=f"c{p}", name=f"c{p}")
                       for p in range(2)]
                for p in range(2):
                    nc.vector.memset(cst[p][:], 0.0)

                for _rep in range(rep):
                  with tc.For_i(
                    0, t2, U, hint_engines=(mybir.EngineType.PE,)
                  ) as iv:
                    xpb = xprow_pool.tile([S, U, FH], bf16, tag="xpb")
                    nc.sync.dma_start(
                        xpb[:], xp_d[ds(iv, U), :, :].rearrange(
                            "u s f -> s u f")
                    )
                    for u in range(U):
                        par = u % 2  # c ping-pong (U even)
                        h_rd = hsb[:, (u - 1) % U, :]
                        h_wr = hsb[:, u, :]
                        c_rd, c_wr = cst[par], cst[1 - par]

                        ps_if = psif.tile([128, 16 * S], f32, tag="if")
                        ps_g = psg.tile([128, 8 * S], f32, tag="g")
                        ps_o = pso.tile([128, 8 * S], f32, tag="o")

                        def gate_group(ps, col, g):
                            nc.tensor.matmul(
                                ps[:, S * col:S * (col + 1)],
                                xpb[0:S, u, g * 128:(g + 1) * 128],
                                eyeS[0:S, 0:S],
                                start=True, stop=False,
                            )
                            for k in range(KB):
                                nc.tensor.matmul(
                                    ps[:, S * col:S * (col + 1)],
                                    whh[k][:, g * 128:(g + 1) * 128],
                                    h_rd[:, S * k:S * (k + 1)],
                                    start=False, stop=(k == KB - 1),
                                )

                        # gate chunks: i (0..7), f (8..15) -> ps_if
                        for g in range(16):
                            gate_group(ps_if, g, g)
                        # g~ (16..23) -> ps_g
                        for g in range(8):
                            gate_group(ps_g, g, 16 + g)
                        # o (24..31) -> ps_o
                        for g in range(8):
                            gate_group(ps_o, g, 24 + g)

                        ift = cell.tile([128, 16 * S], f32, tag="if")
                        nc.scalar.activation(ift[:], ps_if[:], AF.Sigmoid)
                        gt = cell.tile([128, 8 * S], f32, tag="g")
                        nc.scalar.activation(gt[:], ps_g[:], AF.Tanh)
                        ig = cell.tile([128, 8 * S], f32, tag="ig")
                        nc.vector.tensor_mul(ig[:], ift[:, 0:8 * S], gt[:])
                        fc_ = cell.tile([128, 8 * S], f32, tag="fc")
                        nc.vector.tensor_mul(fc_[:], ift[:, 8 * S:16 * S], c_rd[:])
                        nc.vector.tensor_add(c_wr[:], ig[:], fc_[:])
                        tcl = cell.tile([128, 8 * S], f32, tag="tc")
                        nc.scalar.activation(tcl[:], c_wr[:], AF.Tanh)
                        ot = cell.tile([128, 8 * S], f32, tag="o")
                        nc.scalar.activation(ot[:], ps_o[:], AF.Sigmoid)
                        nc.vector.tensor_mul(h_wr[:], ot[:], tcl[:])
                    nc.sync.dma_start(
                        hs_d[ds(iv, U), :, :].rearrange("u p c -> p u c"),
                        hsb[:],
                    )

            tc.strict_bb_all_engine_barrier()

            # ------------- phase 3: out = hs[W:] @ fcWT ---------------------
            with tc.tile_pool(name="p3w", bufs=1) as p3w, \
                 tc.tile_pool(name="p3", bufs=3) as p3, \
                 tc.tile_pool(name="p3ps", bufs=2, space="PSUM") as p3ps, \
                 tc.tile_pool(name="p3tp", bufs=2, space="PSUM") as p3tp:
                fcw = []
                for k in range(KB):
                    wk = p3w.tile([128, C], bf16, tag=f"fcw{k}")
                    nc.sync.dma_start(wk[:], fcWT_d[k * 128:(k + 1) * 128, :])
                    fcw.append(wk)
                # CH = 64: one 64-row output block per stream
                hrow = p3.tile([CH, 128, KB * S], bf16, tag="hrow")
                nc.sync.dma_start(hrow[:], hs_d[W:W + CH, :, :])
                for s in range(S):
                    hTt = []
                    for k in range(KB):
                        tp = p3tp.tile([128, CH], bf16, tag="tr")
                        nc.tensor.transpose(
                            tp[:], hrow[:, :, S * k + s], ident[0:CH, 0:CH]
                        )
                        hk = p3.tile([128, CH], bf16, tag=f"hT{k}")
                        nc.scalar.copy(hk[:], tp[:])
                        hTt.append(hk)
                    for n0, nsz in ((0, 512), (512, C - 512)):
                        ps = p3ps.tile([CH, nsz], f32, tag="ps")
                        for k in range(KB):
                            nc.tensor.matmul(
                                ps[:], hTt[k][:], fcw[k][:, n0:n0 + nsz],
                                start=(k == 0), stop=(k == KB - 1),
                            )
                        ob = p3.tile([CH, nsz], f32, tag="ob")
                        nc.scalar.copy(ob[:], ps[:])
                        nc.sync.dma_start(
                            out_d[s * CH:(s + 1) * CH, n0:n0 + nsz], ob[:]
                        )
    _split_waits(nc)
    return nc


def _get_nc(t2=T2, rep=1):
    if (t2, rep) not in _CACHE:
        _CACHE[(t2, rep)] = _build(t2, rep)
    return _CACHE[(t2, rep)]


def _bf16():
    import ml_dtypes

    return ml_dtypes.bfloat16


def make_in_maps(x, W_ih_f, W_hh_f, bias_f, W_ih_b, W_hh_b, bias_b, fc_W):
    f = _bf16()
    ones = np.ones((1, 128), f)
    eyeS = np.eye(S, dtype=np.float32).astype(f)

    def seg(xd, c):
        """Rows of chunk c incl. warm-up prefix; zero prefix for chunk 0."""
        s = c * CH
        if c == 0:
            return np.concatenate(
                [np.zeros((W, I), xd.dtype), xd[:CH]], axis=0
            )
        return xd[s - W:s + CH]

    def dir_inputs(xd, wih, whh, bias, fcw):
        wihT = np.ascontiguousarray(wih.T).astype(f)
        whhT = np.ascontiguousarray(whh.T).astype(f)
        fcWT = np.ascontiguousarray(fcw.T).astype(f)
        b = bias.reshape(1, FH).astype(f)
        zb = np.zeros((1, FH), f)
        maps = []
        for j in range(NCH // S):
            cA = S * j
            xseg = np.concatenate(
                [seg(xd, cA + s) for s in range(S)], axis=0
            )
            maps.append({
                "xT": np.ascontiguousarray(xseg.T).astype(f),
                "wihT": wihT,
                "bias0A": zb if cA == 0 else b,
                "bias": b,
                "whhT": whhT,
                "fcWT": fcWT,
                "ones1": ones,
                "eyeS": eyeS,
            })
        return maps

    fwd = dir_inputs(x, W_ih_f, W_hh_f, bias_f, fc_W[:, :H])
    bwd = dir_inputs(x[::-1], W_ih_b, W_hh_b, bias_b, fc_W[:, H:])
    return fwd + bwd


def make_in_maps_from_inputs(inputs):
    return make_in_maps(
        np.asarray(inputs["x"], np.float32),
        np.asarray(inputs["W_ih_f"], np.float32),
        np.asarray(inputs["W_hh_f"], np.float32),
        np.asarray(inputs["b_ih_f"], np.float32)
        + np.asarray(inputs["b_hh_f"], np.float32),
        np.asarray(inputs["W_ih_b"], np.float32),
        np.asarray(inputs["W_hh_b"], np.float32),
        np.asarray(inputs["b_ih_b"], np.float32)
        + np.asarray(inputs["b_hh_b"], np.float32),
        np.asarray(inputs["fc_W"], np.float32),
    )


def assemble(results, fc_b):
    nc_half = NCH // S
    out_f = np.concatenate([results[j]["out"] for j in range(nc_half)], axis=0)
    out_b = np.concatenate([results[nc_half + j]["out"]
                            for j in range(nc_half)], axis=0)[::-1]
    return (out_f + out_b + np.asarray(fc_b, np.float32)).astype(np.float32)


def kernel(x, W_ih_f, W_hh_f, b_ih_f, b_hh_f, W_ih_b, W_hh_b, b_ih_b, b_hh_b,
           fc_W, fc_b):
    from concourse.bass_utils import run_bass_kernel_spmd

    nc = _get_nc()
    in_maps = make_in_maps_from_inputs({
        "x": x, "W_ih_f": W_ih_f, "W_hh_f": W_hh_f,
        "b_ih_f": b_ih_f, "b_hh_f": b_hh_f,
        "W_ih_b": W_ih_b, "W_hh_b": W_hh_b,
        "b_ih_b": b_ih_b, "b_hh_b": b_hh_b,
        "fc_W": fc_W,
    })
    res = run_bass_kernel_spmd(nc, in_maps, core_ids=list(range(8)))
    return assemble(res.results, fc_b)
